# revision 3
# baseline (speedup 1.0000x reference)
"""Trainium2 Bass kernel for batched nearest-neighbor min-distance.

Problem: for each row u of U_z [16384, 256], compute
    min_{l in L_z [8192, 256]} ||u - l||_2
Strategy (8 NeuronCores, data-parallel over rows of U_z, L_z replicated;
`pred` is unused by the reference and ignored):
  d2(u,l) = ||u||^2 + ||l||^2 - 2 u.l
v2: fp8(e4m3) DoubleRow matmuls (2x the bf16 PE rate; K=256 packs into a
single 128x256 virtual-array matmul) + a two-engine PSUM consumer:
  Per core (2048 U rows = "columns" of the PSUM tile):
    - SBUF holds L^T [128, 2, 8192] and (-2 U)^T [128, 2, 2048] in e4m3
      (DoubleRow layout: logical K = i*128 + partition for half i).
    - Loop over 64 L-tiles (128 L rows each):
        PSUM[128 Lrows, 2048 Ucols] = (-2 U L^T)^T via 4 DoubleRow matmuls
        (FD=512 each = one PSUM bank, K=256 in one instruction).
        Consumer splits the 2048 columns between the two engines that can
        read PSUM:
          ACT  [0:As]      conv = fp16(psum + l2c)  (1x @ 1.2 GHz)
          DVE  [As:2048]   rmin = min(psum + l2c, rmin) fused stt (1x @ .96)
          DVE  merge       rminA = min(rminA, conv) fp16 TT (2x @ .96)
        merge_span=2 keeps two parity running-min buffers for the ACT share
        so each merge TT covers two tiles' conv output in one op.
    - Partition reduction via DVE 32x32 block transpose + blocked free-dim
      min + two DMA-realigned tree levels, then add ||u||^2 + C, clamp at
      0, sqrt, DMA out [32, 64] fp32 (column c = 32b + i at [i, b]).
The C=256 shift centers l2 so fp16 intermediates keep precision; e4m3
input quantization dominates the error (~9e-3 max rel, gate is 2e-2).
"""

import numpy as np

N, M, D = 16384, 8192, 256
CORES = 8
C_SHIFT = 256.0

ACT_SHARE = 1600   # U columns consumed by ACT conv (rest: DVE fused stt)
MERGE_SPAN = 2     # 1: merge each conv; 2: parity buffers, one TT per 2 tiles

_COMPILED = {}


def _build(ucols: int, m: int, pattern=None, debug: bool = False, rounds: int = 1,
           act_share: int = ACT_SHARE, merge_span: int = MERGE_SPAN,
           conv_bufs: int = 4):
    """Build + compile the per-core Bass kernel.

    ucols:  number of U columns (rows of U_z) this core handles.
    m:      number of L rows (library size).
    rounds: repeat the whole computation this many times inside a hardware
            loop (benchmarking only -- slope between round counts isolates
            steady-state HW time from the host dispatch overhead).
    """
    from contextlib import ExitStack, nullcontext

    import concourse.bacc as bacc
    import concourse.tile as tile
    from concourse import mybir

    F32 = mybir.dt.float32
    F16 = mybir.dt.float16
    FP8 = mybir.dt.float8e4
    AF = mybir.ActivationFunctionType
    ALU = mybir.AluOpType
    DR = mybir.MatmulPerfMode.DoubleRow

    ltiles = m // 128
    As = act_share
    Ds = ucols - As
    assert ucols % 512 == 0 and m % 128 == 0
    assert As % 32 == 0 and Ds % 32 == 0 and 0 < As < ucols
    assert ltiles % 2 == 0

    nc = bacc.Bacc("TRN2", target_bir_lowering=False, debug=debug)

    blocks = ucols // 32
    ut_d = nc.dram_tensor("ut", [128, 2, ucols], FP8, kind="ExternalInput").ap()
    lt_d = nc.dram_tensor("lt", [128, 2, m], FP8, kind="ExternalInput").ap()
    l2c_d = nc.dram_tensor("l2c", [128, ltiles], F32, kind="ExternalInput").ap()
    u2c_d = nc.dram_tensor("u2c", [32, blocks], F32, kind="ExternalInput").ap()
    out_d = nc.dram_tensor("out", [32, blocks], F32, kind="ExternalOutput").ap()

    with tile.TileContext(nc) as tc, ExitStack() as ctx:
        const_pool = ctx.enter_context(tc.tile_pool(name="const", bufs=1))
        psum_pool = ctx.enter_context(
            tc.tile_pool(name="psum", bufs=2, space="PSUM"))
        conv_pool = ctx.enter_context(tc.tile_pool(name="conv", bufs=conv_bufs))

        ut_sb = const_pool.tile([128, 2, ucols], FP8, name="utsb")
        lt_sb = const_pool.tile([128, 2, m], FP8, name="ltsb")
        l2c = const_pool.tile([128, ltiles], F32, name="l2c")
        u2c = const_pool.tile([32, blocks], F32, name="u2c")
        # ACT-share running min: merge_span parity copies side by side.
        rminA = const_pool.tile([128, merge_span * As], F16, name="rminA")
        # DVE-share running min (fused stt from PSUM).
        rminD = const_pool.tile([128, Ds], F16, name="rminD")

        loop_cm = tc.For_i(0, rounds, 1) if rounds > 1 else nullcontext()
        ctx.enter_context(loop_cm)

        # Small + U loads first so the main loop can start on L-chunk 0.
        nc.sync.dma_start(l2c[:], l2c_d[:])
        nc.sync.dma_start(u2c[:], u2c_d[:])
        nc.sync.dma_start(ut_sb[:], ut_d[:])
        CH = min(1024, m)
        for ci, c0 in enumerate(range(0, m, CH)):
            eng = nc.scalar if ci % 2 else nc.sync
            eng.dma_start(lt_sb[:, :, c0:c0 + CH], lt_d[:, :, c0:c0 + CH])

        nc.vector.memset(rminD[:], 60000.0)

        conv2 = None
        for lt in range(ltiles):
            bias = l2c[:, lt:lt + 1]
            psum = psum_pool.tile([128, ucols], F32, name="psum", tag="psum")
            lhsT = lt_sb[:, :, lt * 128:(lt + 1) * 128]
            for s0 in range(0, ucols, 512):
                nc.tensor.matmul(
                    psum[:, s0:s0 + 512],
                    lhsT,
                    ut_sb[:, :, s0:s0 + 512],
                    start=True,
                    stop=True,
                    perf_mode=DR,
                )
            pat = pattern[lt % len(pattern)] if pattern is not None else ""
            if pat == "X":
                continue  # benchmarking variant: no consumer
            if pat == "A!":  # benchmarking: full-width ACT conv only
                conva = conv_pool.tile([128, ucols], F16, name="conva",
                                       tag="conv")
                nc.scalar.activation(conva[:], psum[:], AF.Identity,
                                     bias=bias, scale=1.0)
                continue
            if pat == "D!":  # benchmarking: full-width DVE fused min only
                nc.vector.scalar_tensor_tensor(
                    rminA[:, 0:ucols], psum[:], bias, rminA[:, 0:ucols],
                    op0=ALU.add, op1=ALU.min)
                continue

            # DVE share: fused bias-add + running min straight from PSUM.
            nc.vector.scalar_tensor_tensor(
                rminD[:], psum[:, As:ucols], bias, rminD[:],
                op0=ALU.add, op1=ALU.min)

            # ACT share: convert+bias to fp16; first tile(s) init the
            # running-min parity buffers directly (no merge needed).
            if merge_span == 1:
                if lt == 0:
                    nc.scalar.activation(rminA[:], psum[:, 0:As], AF.Identity,
                                         bias=bias, scale=1.0)
                    continue
                conv = conv_pool.tile([128, As], F16, name="conv", tag="conv")
                nc.scalar.activation(conv[:], psum[:, 0:As], AF.Identity,
                                     bias=bias, scale=1.0)
                nc.vector.tensor_tensor(rminA[:], rminA[:], conv[:],
                                        op=ALU.min)
            else:
                if lt < 2:
                    nc.scalar.activation(rminA[:, lt * As:(lt + 1) * As],
                                         psum[:, 0:As], AF.Identity,
                                         bias=bias, scale=1.0)
                    continue
                par = lt % 2
                if par == 0:
                    conv2 = conv_pool.tile([128, 2 * As], F16, name="conv",
                                           tag="conv")
                nc.scalar.activation(conv2[:, par * As:(par + 1) * As],
                                     psum[:, 0:As], AF.Identity,
                                     bias=bias, scale=1.0)
                if par == 1:
                    nc.vector.tensor_tensor(rminA[:], rminA[:], conv2[:],
                                            op=ALU.min)

        if pattern is not None:
            nc.compile()
            return nc

        # Fold the two parity buffers of the ACT share.
        if merge_span == 2:
            nc.vector.tensor_tensor(rminA[:, 0:As], rminA[:, 0:As],
                                    rminA[:, As:2 * As], op=ALU.min)

        # Partition reduction: transpose every 32x32 block, min over the
        # free dim within each block -> red[32g + i, b] = min over
        # partitions {32g..32g+31} of column 32b + i. Then two tree levels
        # across the four partition groups (base partitions must be
        # 32-aligned and equal for DVE TT, so realign with tiny DMAs).
        tr = const_pool.tile([128, ucols], F16, name="tr")
        nc.vector.transpose(tr[:, 0:As], rminA[:, 0:As])
        nc.vector.transpose(tr[:, As:ucols], rminD[:])
        red = const_pool.tile([128, blocks], F16, name="red")
        nc.vector.tensor_reduce(
            red[:], tr.rearrange("p (b j) -> p b j", j=32),
            axis=mybir.AxisListType.X, op=ALU.min,
        )
        half = const_pool.tile([64, blocks], F16, name="half")
        nc.sync.dma_start(half[:], red[64:128, :])
        nc.vector.tensor_tensor(red[:64, :], red[:64, :], half[:, :], op=ALU.min)
        quart = const_pool.tile([32, blocks], F16, name="quart")
        nc.sync.dma_start(quart[:], red[32:64, :])
        nc.vector.tensor_tensor(red[:32, :], red[:32, :], quart[:, :], op=ALU.min)
        pmin = red[:32, :]
        d2 = const_pool.tile([32, blocks], F32, name="d2")
        nc.vector.tensor_tensor(d2[:], pmin[:], u2c[:], op=ALU.add)
        nc.vector.tensor_scalar_max(d2[:], d2[:], 0.0)
        outt = const_pool.tile([32, blocks], F32, name="outt")
        nc.scalar.activation(outt[:], d2[:], AF.Sqrt)
        nc.sync.dma_start(out_d[:], outt[:])

    nc.compile()
    return nc


def _get_compiled(ucols: int, m: int):
    key = (ucols, m)
    if key not in _COMPILED:
        _COMPILED[key] = _build(ucols, m)
    return _COMPILED[key]


def _prep_inputs(U: np.ndarray, L: np.ndarray):
    """Host-side sharding / layout prep (transpose, -2 scale, norm rows).

    DoubleRow operand layout: tile[p, i, x] = T[i*128 + p, x] for the
    transposed operand T [256, X] (logical K index = i*128 + p).
    """
    import ml_dtypes

    n, d = U.shape
    m = L.shape[0]
    ucols = n // CORES
    FP8 = ml_dtypes.float8_e4m3
    UTm2 = np.ascontiguousarray((-2.0 * U).T).reshape(2, 128, n)
    UTm2 = UTm2.transpose(1, 0, 2)  # [128, 2, n]
    LT = np.ascontiguousarray(L.T).reshape(2, 128, m).transpose(1, 0, 2)
    LT8 = np.ascontiguousarray(LT).astype(FP8)
    l2 = (L.astype(np.float64) ** 2).sum(1).astype(np.float32)
    u2 = (U.astype(np.float64) ** 2).sum(1).astype(np.float32)
    l2cT = np.ascontiguousarray((l2 - C_SHIFT).reshape(m // 128, 128).T)
    u2c = u2 + C_SHIFT
    in_maps = []
    for i in range(CORES):
        sl = slice(i * ucols, (i + 1) * ucols)
        # Device output layout is [32, ucols//32] with column c = 32*b + i at
        # [i, b]; u2c must match that layout.
        u2c_dev = np.ascontiguousarray(u2c[sl].reshape(ucols // 32, 32).T)
        in_maps.append({
            "ut": np.ascontiguousarray(UTm2[:, :, sl]).astype(FP8),
            "lt": LT8,
            "l2c": l2cT,
            "u2c": u2c_dev,
        })
    return in_maps


def kernel(**inputs) -> np.ndarray:
    from concourse import bass_utils

    U = np.asarray(inputs["U_z"], dtype=np.float32)
    L = np.asarray(inputs["L_z"], dtype=np.float32)
    n = U.shape[0]
    m = L.shape[0]
    ucols = n // CORES
    nc = _get_compiled(ucols, m)
    in_maps = _prep_inputs(U, L)
    res = bass_utils.run_bass_kernel_spmd(nc, in_maps, list(range(CORES)))
    # Per-core output [32, ucols//32] holds column c = 32*b + i at [i, b].
    return np.concatenate(
        [np.ascontiguousarray(r["out"].T).reshape(-1) for r in res.results]
    ).astype(np.float32)


if __name__ == "__main__":
    # Smoke test with random data against a numpy reference.
    rng = np.random.default_rng(0)
    U = rng.standard_normal((N, D), dtype=np.float32)
    L = rng.standard_normal((M, D), dtype=np.float32)
    out = kernel(pred=None, U_z=U, L_z=L)
    d2 = (U * U).sum(1)[:, None] + (L * L).sum(1)[None, :] - 2.0 * U @ L.T
    exp = np.sqrt(np.maximum(d2, 0.0).min(1))
    rel = np.abs(out - exp) / np.maximum(np.abs(exp), 1e-9)
    print("max rel err:", rel.max())


# revision 15
# speedup vs baseline: 1.2164x; 1.2164x over previous
"""Trainium2 Bass kernel for batched nearest-neighbor min-distance.

Problem: for each row u of U_z [16384, 256], compute
    min_{l in L_z [8192, 256]} ||u - l||_2
Strategy (8 NeuronCores, data-parallel over rows of U_z, L_z replicated;
`pred` is unused by the reference and ignored):
  d2(u,l) = ||u||^2 + ||l||^2 - 2 u.l
v2: fp8(e4m3) DoubleRow matmuls (2x the bf16 PE rate; K=256 packs into a
single 128x256 virtual-array matmul) + a two-engine PSUM consumer:
  Per core (2048 U rows = "columns" of the PSUM tile):
    - SBUF holds L^T [128, 2, 8192] and (-2 U)^T [128, 2, 2048] in e4m3
      (DoubleRow layout: logical K = i*128 + partition for half i).
    - Loop over 64 L-tiles (128 L rows each):
        PSUM[128 Lrows, 2048 Ucols] = (-2 U L^T)^T via 4 DoubleRow matmuls
        (FD=512 each = one PSUM bank, K=256 in one instruction).
        Consumer splits the 2048 columns between the two engines that can
        read PSUM:
          ACT  [0:As]      conv = fp16(psum + l2c)  (1x @ 1.2 GHz)
          DVE  [As:2048]   rmin = min(psum + l2c, rmin) fused stt (1x @ .96)
          DVE  merge       rminA = min(rminA, conv) fp16 TT (2x @ .96)
        merge_span=2 keeps two parity running-min buffers for the ACT share
        so each merge TT covers two tiles' conv output in one op.
    - Partition reduction via DVE 32x32 block transpose + blocked free-dim
      min + two DMA-realigned tree levels, then add ||u||^2 + C, clamp at
      0, sqrt, DMA out [32, 64] fp32 (column c = 32b + i at [i, b]).
The C=256 shift centers l2 so fp16 intermediates keep precision; e4m3
input quantization dominates the error (~9e-3 max rel, gate is 2e-2).
"""

import numpy as np

N, M, D = 16384, 8192, 256
CORES = 8
C_SHIFT = 256.0

ACT_SHARE = 1600   # U columns consumed by ACT conv (rest: DVE fused stt)
MERGE_SPAN = 2     # 1: merge each conv; 2: parity buffers, one TT per 2 tiles

_COMPILED = {}


def _build(ucols: int, m: int, pattern=None, debug: bool = False, rounds: int = 1,
           act_share: int = ACT_SHARE, merge_span: int = MERGE_SPAN,
           conv_bufs: int = 4, mm_mode: str = "drswi"):
    """Build + compile the per-core Bass kernel.

    ucols:  number of U columns (rows of U_z) this core handles.
    m:      number of L rows (library size).
    rounds: repeat the whole computation this many times inside a hardware
            loop (benchmarking only -- slope between round counts isolates
            steady-state HW time from the host dispatch overhead).
    """
    from contextlib import ExitStack, nullcontext

    import concourse.bacc as bacc
    import concourse.tile as tile
    from concourse import mybir

    F32 = mybir.dt.float32
    F16 = mybir.dt.float16
    FP8 = mybir.dt.float8e4
    AF = mybir.ActivationFunctionType
    ALU = mybir.AluOpType
    # DoubleRowSwInterleave: host pre-interleaves the weight pairs so the
    # per-matmul LDWEIGHTS is a contiguous (FWL-speed) read.  Plain
    # DoubleRow's LDW is a reversed strided read that walrus re-emits
    # before EVERY matmul -- measured ~4.6us/tile, 5x the matmul cost.
    DR = (mybir.MatmulPerfMode.DoubleRowSwInterleave if mm_mode == "drswi"
          else mybir.MatmulPerfMode.DoubleRow)

    ltiles = m // 128
    As = act_share
    Ds = ucols - As
    assert ucols % 512 == 0 and m % 128 == 0
    assert As % 32 == 0 and Ds % 32 == 0 and 0 < As < ucols
    assert ltiles % 2 == 0

    nc = bacc.Bacc("TRN2", target_bir_lowering=False, debug=debug)

    blocks = ucols // 32
    ut_d = nc.dram_tensor("ut", [128, 2, ucols], FP8, kind="ExternalInput").ap()
    # drswi: per L-tile 256 interleaved+reversed weight bytes per partition
    # (see _prep_inputs); dr: [128, 2, m] K-split layout.
    lt_shape = [128, 2 * m] if mm_mode == "drswi" else [128, 2, m]
    lt_d = nc.dram_tensor("lt", lt_shape, FP8, kind="ExternalInput").ap()
    l2c_d = nc.dram_tensor("l2c", [128, ltiles], F32, kind="ExternalInput").ap()
    u2c_d = nc.dram_tensor("u2c", [32, blocks], F32, kind="ExternalInput").ap()
    out_d = nc.dram_tensor("out", [32, blocks], F32, kind="ExternalOutput").ap()

    with tile.TileContext(nc) as tc, ExitStack() as ctx:
        const_pool = ctx.enter_context(tc.tile_pool(name="const", bufs=1))
        psum_pool = ctx.enter_context(
            tc.tile_pool(name="psum", bufs=2, space="PSUM"))
        conv_pool = ctx.enter_context(tc.tile_pool(name="conv", bufs=conv_bufs))

        ut_sb = const_pool.tile([128, 2, ucols], FP8, name="utsb")
        lt_sb = const_pool.tile(lt_shape, FP8, name="ltsb")
        l2c = const_pool.tile([128, ltiles], F32, name="l2c")
        u2c = const_pool.tile([32, blocks], F32, name="u2c")
        # ACT-share running min: merge_span parity copies side by side.
        rminA = const_pool.tile([128, merge_span * As], F16, name="rminA")
        # DVE-share running min (fused stt from PSUM).
        rminD = const_pool.tile([128, Ds], F16, name="rminD")

        loop_cm = tc.For_i(0, rounds, 1) if rounds > 1 else nullcontext()
        ctx.enter_context(loop_cm)

        # Small + U loads first so the main loop can start on L-chunk 0.
        nc.sync.dma_start(l2c[:], l2c_d[:])
        nc.sync.dma_start(u2c[:], u2c_d[:])
        nc.sync.dma_start(ut_sb[:], ut_d[:])
        if mm_mode == "drswi":
            CH = min(2048, 2 * m)
            for ci, c0 in enumerate(range(0, 2 * m, CH)):
                eng = nc.scalar if ci % 2 else nc.sync
                eng.dma_start(lt_sb[:, c0:c0 + CH], lt_d[:, c0:c0 + CH])
        else:
            CH = min(1024, m)
            for ci, c0 in enumerate(range(0, m, CH)):
                eng = nc.scalar if ci % 2 else nc.sync
                eng.dma_start(lt_sb[:, :, c0:c0 + CH], lt_d[:, :, c0:c0 + CH])

        nc.vector.memset(rminD[:], 60000.0)
        if pattern is not None:
            # Benchmarking variants skip some/all consumers; init the
            # running-min buffers so the tail still produces an output.
            nc.vector.memset(rminA[:], 60000.0)

        conv2 = None
        for lt in range(ltiles):
            bias = l2c[:, lt:lt + 1]
            psum = psum_pool.tile([128, ucols], F32, name="psum", tag="psum")
            if mm_mode == "drswi":
                lhsT = lt_sb[:, lt * 256:(lt + 1) * 256]
            else:
                lhsT = lt_sb[:, :, lt * 128:(lt + 1) * 128]
            for s0 in range(0, ucols, 512):
                nc.tensor.matmul(
                    psum[:, s0:s0 + 512],
                    lhsT,
                    ut_sb[:, :, s0:s0 + 512],
                    start=True,
                    stop=True,
                    perf_mode=DR,
                )
            pat = pattern[lt % len(pattern)] if pattern is not None else ""
            if pat == "X":
                continue  # benchmarking variant: no consumer
            if pat == "A!":  # benchmarking: full-width ACT conv only
                conva = conv_pool.tile([128, ucols], F16, name="conva",
                                       tag="conv")
                nc.scalar.activation(conva[:], psum[:], AF.Identity,
                                     bias=bias, scale=1.0)
                continue
            if pat == "D!":  # benchmarking: full-width DVE fused min only
                nc.vector.scalar_tensor_tensor(
                    rminA[:, 0:ucols], psum[:], bias, rminA[:, 0:ucols],
                    op0=ALU.add, op1=ALU.min)
                continue

            # DVE share: fused bias-add + running min straight from PSUM.
            nc.vector.scalar_tensor_tensor(
                rminD[:], psum[:, As:ucols], bias, rminD[:],
                op0=ALU.add, op1=ALU.min)

            # ACT share: convert+bias to fp16; first tile(s) init the
            # running-min parity buffers directly (no merge needed).
            if merge_span == 1:
                if lt == 0:
                    nc.scalar.activation(rminA[:], psum[:, 0:As], AF.Identity,
                                         bias=bias, scale=1.0)
                    continue
                conv = conv_pool.tile([128, As], F16, name="conv", tag="conv")
                nc.scalar.activation(conv[:], psum[:, 0:As], AF.Identity,
                                     bias=bias, scale=1.0)
                nc.vector.tensor_tensor(rminA[:], rminA[:], conv[:],
                                        op=ALU.min)
            else:
                if lt < 2:
                    nc.scalar.activation(rminA[:, lt * As:(lt + 1) * As],
                                         psum[:, 0:As], AF.Identity,
                                         bias=bias, scale=1.0)
                    continue
                par = lt % 2
                if par == 0:
                    conv2 = conv_pool.tile([128, 2 * As], F16, name="conv",
                                           tag="conv")
                nc.scalar.activation(conv2[:, par * As:(par + 1) * As],
                                     psum[:, 0:As], AF.Identity,
                                     bias=bias, scale=1.0)
                if par == 1:
                    nc.vector.tensor_tensor(rminA[:], rminA[:], conv2[:],
                                            op=ALU.min)

        # Fold the two parity buffers of the ACT share.
        if merge_span == 2:
            nc.vector.tensor_tensor(rminA[:, 0:As], rminA[:, 0:As],
                                    rminA[:, As:2 * As], op=ALU.min)

        # Partition reduction: transpose every 32x32 block, min over the
        # free dim within each block -> red[32g + i, b] = min over
        # partitions {32g..32g+31} of column 32b + i. Then two tree levels
        # across the four partition groups (base partitions must be
        # 32-aligned and equal for DVE TT, so realign with tiny DMAs).
        tr = const_pool.tile([128, ucols], F16, name="tr")
        nc.vector.transpose(tr[:, 0:As], rminA[:, 0:As])
        nc.vector.transpose(tr[:, As:ucols], rminD[:])
        red = const_pool.tile([128, blocks], F16, name="red")
        nc.vector.tensor_reduce(
            red[:], tr.rearrange("p (b j) -> p b j", j=32),
            axis=mybir.AxisListType.X, op=ALU.min,
        )
        half = const_pool.tile([64, blocks], F16, name="half")
        nc.sync.dma_start(half[:], red[64:128, :])
        nc.vector.tensor_tensor(red[:64, :], red[:64, :], half[:, :], op=ALU.min)
        quart = const_pool.tile([32, blocks], F16, name="quart")
        nc.sync.dma_start(quart[:], red[32:64, :])
        nc.vector.tensor_tensor(red[:32, :], red[:32, :], quart[:, :], op=ALU.min)
        pmin = red[:32, :]
        d2 = const_pool.tile([32, blocks], F32, name="d2")
        nc.vector.tensor_tensor(d2[:], pmin[:], u2c[:], op=ALU.add)
        nc.vector.tensor_scalar_max(d2[:], d2[:], 0.0)
        outt = const_pool.tile([32, blocks], F32, name="outt")
        nc.scalar.activation(outt[:], d2[:], AF.Sqrt)
        nc.sync.dma_start(out_d[:], outt[:])

    nc.compile()
    return nc


def _get_compiled(ucols: int, m: int):
    key = (ucols, m)
    if key not in _COMPILED:
        _COMPILED[key] = _build(ucols, m)
    return _COMPILED[key]


def _prep_inputs(U: np.ndarray, L: np.ndarray, mm_mode: str = "drswi"):
    """Host-side sharding / layout prep (transpose, -2 scale, norm rows).

    Moving operand (U) DoubleRow layout: tile[p, i, x] = T[i*128 + p, x]
    for the transposed operand T [256, X] (logical K index = i*128 + p).
    Stationary operand (L) for DoubleRowSwInterleave: per L-tile, 256
    bytes per partition with w[p, 2*j + i] = LT[i*128 + p, tile*128 +
    (127 - j)] (pairs interleaved per column, columns reversed), so the
    hardware LDWEIGHTS is a contiguous read.
    """
    import ml_dtypes

    n, d = U.shape
    m = L.shape[0]
    ucols = n // CORES
    FP8 = ml_dtypes.float8_e4m3
    UTm2 = np.ascontiguousarray((-2.0 * U).T).reshape(2, 128, n)
    UTm2 = UTm2.transpose(1, 0, 2)  # [128, 2, n]
    LT3 = np.ascontiguousarray(L.T).reshape(2, 128, m)  # [i, p, dcol]
    if mm_mode == "drswi":
        # [i, p, tile, j'] with column reversal inside each 128-wide tile
        B = LT3.reshape(2, 128, m // 128, 128)[:, :, :, ::-1]
        # -> [p, tile, j', i] -> flatten to [128, 2*m]
        LT8 = np.ascontiguousarray(
            B.transpose(1, 2, 3, 0).reshape(128, 2 * m)).astype(FP8)
    else:
        LT8 = np.ascontiguousarray(LT3.transpose(1, 0, 2)).astype(FP8)
    l2 = (L.astype(np.float64) ** 2).sum(1).astype(np.float32)
    u2 = (U.astype(np.float64) ** 2).sum(1).astype(np.float32)
    l2cT = np.ascontiguousarray((l2 - C_SHIFT).reshape(m // 128, 128).T)
    u2c = u2 + C_SHIFT
    in_maps = []
    for i in range(CORES):
        sl = slice(i * ucols, (i + 1) * ucols)
        # Device output layout is [32, ucols//32] with column c = 32*b + i at
        # [i, b]; u2c must match that layout.
        u2c_dev = np.ascontiguousarray(u2c[sl].reshape(ucols // 32, 32).T)
        in_maps.append({
            "ut": np.ascontiguousarray(UTm2[:, :, sl]).astype(FP8),
            "lt": LT8,
            "l2c": l2cT,
            "u2c": u2c_dev,
        })
    return in_maps


def kernel(**inputs) -> np.ndarray:
    from concourse import bass_utils

    U = np.asarray(inputs["U_z"], dtype=np.float32)
    L = np.asarray(inputs["L_z"], dtype=np.float32)
    n = U.shape[0]
    m = L.shape[0]
    ucols = n // CORES
    nc = _get_compiled(ucols, m)
    in_maps = _prep_inputs(U, L)
    res = bass_utils.run_bass_kernel_spmd(nc, in_maps, list(range(CORES)))
    # Per-core output [32, ucols//32] holds column c = 32*b + i at [i, b].
    return np.concatenate(
        [np.ascontiguousarray(r["out"].T).reshape(-1) for r in res.results]
    ).astype(np.float32)


if __name__ == "__main__":
    # Smoke test with random data against a numpy reference.
    rng = np.random.default_rng(0)
    U = rng.standard_normal((N, D), dtype=np.float32)
    L = rng.standard_normal((M, D), dtype=np.float32)
    out = kernel(pred=None, U_z=U, L_z=L)
    d2 = (U * U).sum(1)[:, None] + (L * L).sum(1)[None, :] - 2.0 * U @ L.T
    exp = np.sqrt(np.maximum(d2, 0.0).min(1))
    rel = np.abs(out - exp) / np.maximum(np.abs(exp), 1e-9)
    print("max rel err:", rel.max())


# revision 20
# speedup vs baseline: 6.0999x; 5.0148x over previous
"""Trainium2 Bass kernel for batched nearest-neighbor min-distance.

Problem: for each row u of U_z [16384, 256], compute
    min_{l in L_z [8192, 256]} ||u - l||_2
Strategy (8 NeuronCores, data-parallel over rows of U_z, L_z replicated;
`pred` is unused by the reference and ignored):
  d2(u,l) = ||u||^2 + ||l||^2 - 2 u.l
v2: fp8(e4m3) DoubleRow matmuls (2x the bf16 PE rate; K=256 packs into a
single 128x256 virtual-array matmul) + a two-engine PSUM consumer:
  Per core (2048 U rows = "columns" of the PSUM tile):
    - SBUF holds L^T [128, 2, 8192] and (-2 U)^T [128, 2, 2048] in e4m3
      (DoubleRow layout: logical K = i*128 + partition for half i).
    - Loop over 64 L-tiles (128 L rows each):
        PSUM[128 Lrows, 2048 Ucols] = (-2 U L^T)^T via 4 DoubleRow matmuls
        (FD=512 each = one PSUM bank, K=256 in one instruction).
        Consumer splits the 2048 columns between the two engines that can
        read PSUM:
          ACT  [0:As]      conv = fp16(psum + l2c)  (1x @ 1.2 GHz)
          DVE  [As:2048]   rmin = min(psum + l2c, rmin) fused stt (1x @ .96)
          DVE  merge       rminA = min(rminA, conv) fp16 TT (2x @ .96)
        merge_span=2 keeps two parity running-min buffers for the ACT share
        so each merge TT covers two tiles' conv output in one op.
    - Partition reduction via DVE 32x32 block transpose + blocked free-dim
      min + two DMA-realigned tree levels, then add ||u||^2 + C, clamp at
      0, sqrt, DMA out [32, 64] fp32 (column c = 32b + i at [i, b]).
The C=256 shift centers l2 so fp16 intermediates keep precision; e4m3
input quantization dominates the error (~9e-3 max rel, gate is 2e-2).
"""

import numpy as np

N, M, D = 16384, 8192, 256
CORES = 8
C_SHIFT = 256.0

ACT_SHARE = 1600   # U columns consumed by ACT conv (rest: DVE fused stt)
MERGE_SPAN = 2     # 1: merge each conv; 2: parity buffers, one TT per 2 tiles

_COMPILED = {}


def _build(ucols: int, m: int, pattern=None, debug: bool = False, rounds: int = 1,
           act_share: int = ACT_SHARE, merge_span: int = MERGE_SPAN,
           conv_bufs: int = 4, mm_mode: str = "drswi"):
    """Build + compile the per-core Bass kernel.

    ucols:  number of U columns (rows of U_z) this core handles.
    m:      number of L rows (library size).
    rounds: repeat the whole computation this many times inside a hardware
            loop (benchmarking only -- slope between round counts isolates
            steady-state HW time from the host dispatch overhead).
    """
    from contextlib import ExitStack, nullcontext

    import concourse.bacc as bacc
    import concourse.tile as tile
    from concourse import mybir

    F32 = mybir.dt.float32
    F16 = mybir.dt.float16
    FP8 = mybir.dt.float8e4
    AF = mybir.ActivationFunctionType
    ALU = mybir.AluOpType
    # DoubleRowSwInterleave: host pre-interleaves the weight pairs so the
    # per-matmul LDWEIGHTS is a contiguous (FWL-speed) read.  Plain
    # DoubleRow's LDW is a reversed strided read that walrus re-emits
    # before EVERY matmul -- measured ~4.6us/tile, 5x the matmul cost.
    DR = (mybir.MatmulPerfMode.DoubleRowSwInterleave if mm_mode == "drswi"
          else mybir.MatmulPerfMode.DoubleRow)

    ltiles = m // 128
    As = act_share
    Ds = ucols - As
    assert ucols % 512 == 0 and m % 128 == 0
    assert As % 32 == 0 and Ds % 32 == 0 and 0 < As < ucols
    assert ltiles % 2 == 0

    nc = bacc.Bacc("TRN2", target_bir_lowering=False, debug=debug)

    blocks = ucols // 32
    ut_d = nc.dram_tensor("ut", [128, 2, ucols], FP8, kind="ExternalInput").ap()
    # drswi: per L-tile 256 interleaved+reversed weight bytes per partition
    # (see _prep_inputs); dr: [128, 2, m] K-split layout.
    lt_shape = [128, 2 * m] if mm_mode == "drswi" else [128, 2, m]
    lt_d = nc.dram_tensor("lt", lt_shape, FP8, kind="ExternalInput").ap()
    l2c_d = nc.dram_tensor("l2c", [128, ltiles], F32, kind="ExternalInput").ap()
    u2c_d = nc.dram_tensor("u2c", [32, blocks], F32, kind="ExternalInput").ap()
    out_d = nc.dram_tensor("out", [32, blocks], F32, kind="ExternalOutput").ap()

    with tile.TileContext(nc) as tc, ExitStack() as ctx:
        const_pool = ctx.enter_context(tc.tile_pool(name="const", bufs=1))
        psum_pool = ctx.enter_context(
            tc.tile_pool(name="psum", bufs=2, space="PSUM"))
        conv_pool = ctx.enter_context(tc.tile_pool(name="conv", bufs=conv_bufs))

        ut_sb = const_pool.tile([128, 2, ucols], FP8, name="utsb")
        lt_sb = const_pool.tile(lt_shape, FP8, name="ltsb")
        l2c = const_pool.tile([128, ltiles], F32, name="l2c")
        u2c = const_pool.tile([32, blocks], F32, name="u2c")
        # Per-engine running mins (tile-alternating consumers).
        rminA = const_pool.tile([128, ucols], F16, name="rminA")
        rminD = const_pool.tile([128, ucols], F16, name="rminD")

        loop_cm = tc.For_i(0, rounds, 1) if rounds > 1 else nullcontext()
        ctx.enter_context(loop_cm)

        # Small + U loads first so the main loop can start on L-chunk 0.
        nc.sync.dma_start(l2c[:], l2c_d[:])
        nc.sync.dma_start(u2c[:], u2c_d[:])
        nc.sync.dma_start(ut_sb[:], ut_d[:])
        if mm_mode == "drswi":
            CH = min(2048, 2 * m)
            for c0 in range(0, 2 * m, CH):
                nc.sync.dma_start(lt_sb[:, c0:c0 + CH], lt_d[:, c0:c0 + CH])
        else:
            CH = min(1024, m)
            for c0 in range(0, m, CH):
                nc.sync.dma_start(lt_sb[:, :, c0:c0 + CH], lt_d[:, :, c0:c0 + CH])

        nc.vector.memset(rminD[:], 60000.0)
        if pattern is not None:
            # Benchmarking variants skip some/all consumers; init the
            # running-min buffers so the tail still produces an output.
            nc.vector.memset(rminA[:], 60000.0)

        act_tiles_left = 45
        for lt in range(ltiles):
            bias = l2c[:, lt:lt + 1]
            psum = psum_pool.tile([128, ucols], F32, name="psum", tag="psum")
            if mm_mode == "drswi":
                lhsT = lt_sb[:, lt * 256:(lt + 1) * 256]
            else:
                lhsT = lt_sb[:, :, lt * 128:(lt + 1) * 128]
            for s0 in range(0, ucols, 512):
                nc.tensor.matmul(
                    psum[:, s0:s0 + 512],
                    lhsT,
                    ut_sb[:, :, s0:s0 + 512],
                    start=True,
                    stop=True,
                    perf_mode=DR,
                )
            pat = pattern[lt % len(pattern)] if pattern is not None else ""
            if pat == "X":
                continue  # benchmarking variant: no consumer
            if pat == "A!":  # benchmarking: full-width ACT conv only
                conva = conv_pool.tile([128, ucols], F16, name="conva",
                                       tag="conv")
                nc.scalar.activation(conva[:], psum[:], AF.Identity,
                                     bias=bias, scale=1.0)
                continue
            if pat == "D!":  # benchmarking: full-width DVE fused min only
                nc.vector.scalar_tensor_tensor(
                    rminA[:, 0:ucols], psum[:], bias, rminA[:, 0:ucols],
                    op0=ALU.add, op1=ALU.min)
                continue

            # Tile-alternating consumers: each PSUM region is read by
            # exactly ONE engine (no shared banks, single turnaround op).
            # 45:19 ACT:DVE tile ratio balances ACT convs (2.1us) against
            # DVE stt (2.5us) + merge (1.1us) duty.
            if lt % 7 < 5 and act_tiles_left > 0:
                act_tiles_left -= 1
                if lt == 0:
                    nc.scalar.activation(rminA[:, 0:ucols], psum[:],
                                         AF.Identity, bias=bias, scale=1.0)
                    continue
                conv = conv_pool.tile([128, ucols], F16, name="conv",
                                      tag="conv")
                nc.scalar.activation(conv[:], psum[:], AF.Identity,
                                     bias=bias, scale=1.0)
                nc.vector.tensor_tensor(rminA[:, 0:ucols],
                                        rminA[:, 0:ucols], conv[:],
                                        op=ALU.min)
            else:
                # DVE: fused bias-add + running min straight from PSUM.
                nc.vector.scalar_tensor_tensor(
                    rminD[:], psum[:], bias, rminD[:],
                    op0=ALU.add, op1=ALU.min)

        # Fold the two engines' running mins.
        nc.vector.tensor_tensor(rminA[:, 0:ucols], rminA[:, 0:ucols],
                                rminD[:], op=ALU.min)

        # Partition reduction: transpose every 32x32 block, min over the
        # free dim within each block -> red[32g + i, b] = min over
        # partitions {32g..32g+31} of column 32b + i. Then two tree levels
        # across the four partition groups (base partitions must be
        # 32-aligned and equal for DVE TT, so realign with tiny DMAs).
        tr = const_pool.tile([128, ucols], F16, name="tr")
        nc.vector.transpose(tr[:], rminA[:, 0:ucols])
        red = const_pool.tile([128, blocks], F16, name="red")
        nc.vector.tensor_reduce(
            red[:], tr.rearrange("p (b j) -> p b j", j=32),
            axis=mybir.AxisListType.X, op=ALU.min,
        )
        half = const_pool.tile([64, blocks], F16, name="half")
        nc.sync.dma_start(half[:], red[64:128, :])
        nc.vector.tensor_tensor(red[:64, :], red[:64, :], half[:, :], op=ALU.min)
        quart = const_pool.tile([32, blocks], F16, name="quart")
        nc.sync.dma_start(quart[:], red[32:64, :])
        nc.vector.tensor_tensor(red[:32, :], red[:32, :], quart[:, :], op=ALU.min)
        pmin = red[:32, :]
        d2 = const_pool.tile([32, blocks], F32, name="d2")
        nc.vector.tensor_tensor(d2[:], pmin[:], u2c[:], op=ALU.add)
        nc.vector.tensor_scalar_max(d2[:], d2[:], 0.0)
        outt = const_pool.tile([32, blocks], F32, name="outt")
        nc.scalar.activation(outt[:], d2[:], AF.Sqrt)
        nc.sync.dma_start(out_d[:], outt[:])

    nc.compile()
    return nc


def _get_compiled(ucols: int, m: int):
    key = (ucols, m)
    if key not in _COMPILED:
        _COMPILED[key] = _build(ucols, m)
    return _COMPILED[key]


def _prep_inputs(U: np.ndarray, L: np.ndarray, mm_mode: str = "drswi"):
    """Host-side sharding / layout prep (transpose, -2 scale, norm rows).

    Moving operand (U) DoubleRow layout: tile[p, i, x] = T[i*128 + p, x]
    for the transposed operand T [256, X] (logical K index = i*128 + p).
    Stationary operand (L) for DoubleRowSwInterleave: per L-tile, 256
    bytes per partition with w[p, 2*j + i] = LT[i*128 + p, tile*128 +
    (127 - j)] (pairs interleaved per column, columns reversed), so the
    hardware LDWEIGHTS is a contiguous read.
    """
    import ml_dtypes

    n, d = U.shape
    m = L.shape[0]
    ucols = n // CORES
    FP8 = ml_dtypes.float8_e4m3
    UTm2 = np.ascontiguousarray((-2.0 * U).T).reshape(2, 128, n)
    UTm2 = UTm2.transpose(1, 0, 2)  # [128, 2, n]
    LT3 = np.ascontiguousarray(L.T).reshape(2, 128, m)  # [i, p, dcol]
    if mm_mode == "drswi":
        # [i, p, tile, j'] with column reversal inside each 128-wide tile
        B = LT3.reshape(2, 128, m // 128, 128)[:, :, :, ::-1]
        # -> [p, tile, j', i] -> flatten to [128, 2*m]
        LT8 = np.ascontiguousarray(
            B.transpose(1, 2, 3, 0).reshape(128, 2 * m)).astype(FP8)
    else:
        LT8 = np.ascontiguousarray(LT3.transpose(1, 0, 2)).astype(FP8)
    l2 = (L.astype(np.float64) ** 2).sum(1).astype(np.float32)
    u2 = (U.astype(np.float64) ** 2).sum(1).astype(np.float32)
    l2cT = np.ascontiguousarray((l2 - C_SHIFT).reshape(m // 128, 128).T)
    u2c = u2 + C_SHIFT
    in_maps = []
    for i in range(CORES):
        sl = slice(i * ucols, (i + 1) * ucols)
        # Device output layout is [32, ucols//32] with column c = 32*b + i at
        # [i, b]; u2c must match that layout.
        u2c_dev = np.ascontiguousarray(u2c[sl].reshape(ucols // 32, 32).T)
        in_maps.append({
            "ut": np.ascontiguousarray(UTm2[:, :, sl]).astype(FP8),
            "lt": LT8,
            "l2c": l2cT,
            "u2c": u2c_dev,
        })
    return in_maps


def kernel(**inputs) -> np.ndarray:
    from concourse import bass_utils

    U = np.asarray(inputs["U_z"], dtype=np.float32)
    L = np.asarray(inputs["L_z"], dtype=np.float32)
    n = U.shape[0]
    m = L.shape[0]
    ucols = n // CORES
    nc = _get_compiled(ucols, m)
    in_maps = _prep_inputs(U, L)
    res = bass_utils.run_bass_kernel_spmd(nc, in_maps, list(range(CORES)))
    # Per-core output [32, ucols//32] holds column c = 32*b + i at [i, b].
    return np.concatenate(
        [np.ascontiguousarray(r["out"].T).reshape(-1) for r in res.results]
    ).astype(np.float32)


if __name__ == "__main__":
    # Smoke test with random data against a numpy reference.
    rng = np.random.default_rng(0)
    U = rng.standard_normal((N, D), dtype=np.float32)
    L = rng.standard_normal((M, D), dtype=np.float32)
    out = kernel(pred=None, U_z=U, L_z=L)
    d2 = (U * U).sum(1)[:, None] + (L * L).sum(1)[None, :] - 2.0 * U @ L.T
    exp = np.sqrt(np.maximum(d2, 0.0).min(1))
    rel = np.abs(out - exp) / np.maximum(np.abs(exp), 1e-9)
    print("max rel err:", rel.max())
